# revision 17
# baseline (speedup 1.0000x reference)
"""Multi-head attention Trainium2 kernel (Bass/Tile), data-parallel over batch.

Problem shapes (hardcoded): x [8, 1024, 1024] fp32, 16 heads x 64 dim,
shared per-head projections Wq/Wk/Wv [64, 64], output proj Wo [1024, 1024].

Reference math (note quirks):
  xh = x reshaped to [h, b, m, d]
  Q/K/V = xh @ W{q,k,v}.T + b
  scores = einsum('hbmd,hbnd->hbmn', K, Q) / sqrt(1024)   (K @ Q^T!)
  A = softmax(scores, axis=-1)
  out = (A @ V) transposed (0,1,3,2) then .reshape(b, m, D) @ Wo.T + bo

Per-core plan (core b handles batch b, no collectives):
  - host prepares xT = x[b].T, blockdiag lhsT weights for 2-head packed
    projections, WoT = Wo.T
  - QT/KT/VT [64*16, m] computed via blockdiag [128,128] matmuls
  - per head: S_T[n, m] = QT.T @ KT (scores transposed); even/odd head
    matmuls interleaved (disjoint PE row groups run concurrently); exp on
    ACT with scale 1/32 (softmax max-subtraction skipped; scores are O(1))
  - U[65, m] = [V | ones].T @ expS  -> row 64 = softmax denominator
  - PE-transpose U -> [m, 65], normalize cols by reciprocal of col 64 -> P.T
  - Y rows for the pair's heads = P.T chunk.T @ WoT, interleaved per pair
    (bo added on host); host scatters Y rows (j = h*64+d) into full output
Matmul dtype configurable: "f32r" (fp22 multiply, ~2.5e-4 rel err) or
"f16" (fp16 multiply, faster weight loads, ~1e-3 rel err).
"""

import os

import numpy as np

B = 8
M = 1024
D = 1024
NT = 8  # 128-row tiles in M / D

DTYPE_MODE = os.environ.get("KERNEL_DTYPE", "f32r")

_compiled = {}


def _build(mode):
    import concourse.bacc as bacc
    import concourse.mybir as mybir
    import concourse.tile as tile
    from concourse.masks import make_identity

    f32 = mybir.dt.float32
    mdt = mybir.dt.float32r if mode == "f32r" else mybir.dt.float16
    Exp = mybir.ActivationFunctionType.Exp

    nc = bacc.Bacc("TRN2", target_bir_lowering=False, debug=False, num_devices=B)

    xT_ap = nc.dram_tensor("xT", [D, M], mdt, kind="ExternalInput").ap()
    woT_ap = nc.dram_tensor("woT", [D, D], mdt, kind="ExternalInput").ap()
    wq_ap = nc.dram_tensor("wq", [128, 128], mdt, kind="ExternalInput").ap()
    wk_ap = nc.dram_tensor("wk", [128, 128], mdt, kind="ExternalInput").ap()
    wv_ap = nc.dram_tensor("wv", [128, 128], mdt, kind="ExternalInput").ap()
    bias_ap = nc.dram_tensor("bias", [128, 3], f32, kind="ExternalInput").ap()
    y_ap = nc.dram_tensor("y", [D, M], f32, kind="ExternalOutput").ap()

    with tile.TileContext(nc) as tc:
        with (
            tc.tile_pool(name="persist", bufs=1) as persist,
            tc.tile_pool(name="qkv", bufs=2) as qkv_pool,
            tc.tile_pool(name="vnat", bufs=2) as vnat_pool,
            tc.tile_pool(name="exps", bufs=(4 if mode == "f16" else 2)) as exps_pool,
            tc.tile_pool(name="usb", bufs=3) as usb_pool,
            tc.tile_pool(name="ysb", bufs=2) as ysb_pool,
            tc.tile_pool(name="rec", bufs=4) as rec_pool,
            tc.tile_pool(name="ps", bufs=1, space="PSUM") as ps_pool,
        ):
            # ---- persistent tiles + loads ----
            xT_all = persist.tile([128, NT * M], mdt)  # tile t at cols t*M
            woT_all = persist.tile([128, NT * D], mdt)
            PT_all = persist.tile([128, NT * D], mdt)  # [m-local, mt*D + h*64+d]
            wq_sb = persist.tile([128, 128], mdt)
            wk_sb = persist.tile([128, 128], mdt)
            wv_sb = persist.tile([128, 128], mdt)
            bias_sb = persist.tile([128, 3], f32)
            identity = persist.tile([128, 128], f32)

            with nc.named_scope("loads"):
                nc.sync.dma_start(wq_sb[:], wq_ap[:])
                nc.sync.dma_start(wk_sb[:], wk_ap[:])
                nc.sync.dma_start(wv_sb[:], wv_ap[:])
                nc.sync.dma_start(bias_sb[:], bias_ap[:])
                for t in range(NT):
                    nc.sync.dma_start(
                        xT_all[:, t * M : (t + 1) * M], xT_ap[t * 128 : (t + 1) * 128, :]
                    )
                for t in range(NT):
                    nc.sync.dma_start(
                        woT_all[:, t * D : (t + 1) * D],
                        woT_ap[t * 128 : (t + 1) * 128, :],
                    )
                make_identity(nc, identity[:])

            # ---- per head-pair: QKV, attention, then that pair's slice of
            # the output projection (keeps PE dense and the tail short) ----
            for t in range(8):
                with nc.named_scope(f"qkv_p{t}"):
                    qT = qkv_pool.tile([128, M], mdt, tag="qT")
                    kT = qkv_pool.tile([128, M], mdt, tag="kT")
                    vT = qkv_pool.tile([128, M], f32, tag="vT")
                    for pi, (w_sb, dest) in enumerate(
                        ((wq_sb, qT), (wk_sb, kT), (wv_sb, vT))
                    ):
                        for mh in range(2):
                            ps = ps_pool.tile([128, 512], f32, tag="w", bufs=2)
                            nc.tensor.matmul(
                                ps[:],
                                w_sb[:],
                                xT_all[:, t * M + mh * 512 : t * M + (mh + 1) * 512],
                                start=True,
                                stop=True,
                            )
                            nc.vector.tensor_scalar_add(
                                dest[:, mh * 512 : (mh + 1) * 512],
                                ps[:],
                                bias_sb[:, pi : pi + 1],
                            )

                    # V natural (both heads) + ones cols:
                    # per nt block of 130: [even 64 | 1 | odd 64 | 1]
                    v_nat = vnat_pool.tile([128, NT * 130], mdt, tag="vn")
                    for g in range(2):
                        pst = ps_pool.tile([128, 512], f32, tag="w", bufs=2)
                        for j in range(4):
                            nt = 4 * g + j
                            nc.tensor.transpose(
                                pst[:, j * 128 : (j + 1) * 128],
                                vT[:, nt * 128 : (nt + 1) * 128],
                                identity[:],
                            )
                        # batched strided copies: 4 blocks at once
                        vdst = v_nat[:, g * 520 : (g + 1) * 520].rearrange(
                            "p (n c) -> p n c", c=130
                        )
                        vsrc = pst[:].rearrange("p (n c) -> p n c", c=128)
                        nc.vector.tensor_copy(vdst[:, :, 0:64], vsrc[:, :, 0:64])
                        nc.vector.tensor_copy(vdst[:, :, 65:129], vsrc[:, :, 64:128])
                        ones_cast = f32 if mode == "f32r" else mdt
                        nc.gpsimd.memset(
                            vdst[:, :, 64:65].bitcast(ones_cast), 1.0
                        )
                        nc.gpsimd.memset(
                            vdst[:, :, 129:130].bitcast(ones_cast), 1.0
                        )

                # attention for both heads of the pair; even/odd score
                # matmuls adjacent -> concurrent on disjoint PE row groups
                u_sbs = []
                for hh in range(2):
                    u_sbs.append(usb_pool.tile([65, M], f32, tag="u", name="u_sb"))
                for mh in range(2):
                    with nc.named_scope(f"attn_p{t}_m{mh}"):
                        expS = [
                            exps_pool.tile([128, NT * 512], mdt, tag="es", name="expS_e"),
                            exps_pool.tile([128, NT * 512], mdt, tag="es", name="expS_o"),
                        ]
                        psU = [
                            ps_pool.tile([65, 512], f32, tag="u", bufs=2, name="psU_e"),
                            ps_pool.tile([65, 512], f32, tag="u", bufs=2, name="psU_o"),
                        ]
                        for ntp in range(4):
                            psS = [
                                ps_pool.tile([128, 1024], f32, tag="s", bufs=2, name="psS_e"),
                                ps_pool.tile([128, 1024], f32, tag="s", bufs=2, name="psS_o"),
                            ]
                            for sub in range(2):
                                nt = 2 * ntp + sub
                                for hh in range(2):
                                    part = hh * 64
                                    nc.tensor.matmul(
                                        psS[hh][:, sub * 512 : (sub + 1) * 512],
                                        qT[
                                            part : part + 64,
                                            nt * 128 : (nt + 1) * 128,
                                        ],
                                        kT[
                                            part : part + 64,
                                            mh * 512 : (mh + 1) * 512,
                                        ],
                                        start=True,
                                        stop=True,
                                    )
                            for hh in range(2):
                                nc.scalar.activation(
                                    expS[hh][:, ntp * 1024 : (ntp + 1) * 1024],
                                    psS[hh][:],
                                    Exp,
                                    scale=1.0 / 32.0,
                                )
                            # AV for these two nt blocks, both heads, right
                            # behind the exp -- keeps PE fed per-ntp
                            for sub in range(2):
                                nt = 2 * ntp + sub
                                for hh in range(2):
                                    o = nt * 130 + hh * 65
                                    nc.tensor.matmul(
                                        psU[hh][:],
                                        v_nat[:, o : o + 65],
                                        expS[hh][:, nt * 512 : (nt + 1) * 512],
                                        start=(nt == 0),
                                        stop=(nt == NT - 1),
                                    )
                        for hh in range(2):
                            nc.vector.tensor_copy(
                                u_sbs[hh][:, mh * 512 : (mh + 1) * 512], psU[hh][:]
                            )
                for hh in range(2):
                    h = 2 * t + hh
                    u_sb = u_sbs[hh]
                    with nc.named_scope(f"norm_h{h}"):
                        pstUs = []
                        rec = rec_pool.tile([128, NT], f32, tag="r")
                        for g in range(2):
                            pstU = ps_pool.tile(
                                [128, 512], f32, tag="w", bufs=2, name="pstU"
                            )
                            pstUs.append(pstU)
                            for j in range(4):
                                mt = 4 * g + j
                                nc.tensor.transpose(
                                    pstU[:, j * 128 : j * 128 + 65],
                                    u_sb[:, mt * 128 : (mt + 1) * 128],
                                    identity[:65, :65],
                                )
                            # gather the 4 denominators (col 64 of each slot)
                            nc.vector.tensor_copy(
                                rec[:, g * 4 : (g + 1) * 4],
                                pstU[:].rearrange("p (n c) -> p n c", c=128)[
                                    :, :, 64:65
                                ].rearrange("p n c -> p (n c)"),
                            )
                        nc.vector.reciprocal(rec[:], rec[:])
                        for g in range(2):
                            for j in range(4):
                                mt = 4 * g + j
                                nc.vector.tensor_scalar_mul(
                                    PT_all[:, mt * D + h * 64 : mt * D + h * 64 + 64],
                                    pstUs[g][:, j * 128 : j * 128 + 64],
                                    rec[:, mt : mt + 1],
                                )

                # output projection rows for this pair (j = 128t .. 128t+127)
                with nc.named_scope(f"final_p{t}"):
                    y_sb = ysb_pool.tile([128, 1024], f32, tag="y")
                    for dh in range(2):
                        psY = ps_pool.tile([128, 512], f32, tag="w", bufs=2)
                        for mt in range(NT):
                            nc.tensor.matmul(
                                psY[:],
                                PT_all[:, mt * D + t * 128 : mt * D + (t + 1) * 128],
                                woT_all[:, mt * D + dh * 512 : mt * D + (dh + 1) * 512],
                                start=(mt == 0),
                                stop=(mt == NT - 1),
                            )
                        nc.vector.tensor_copy(
                            y_sb[:, dh * 512 : (dh + 1) * 512], psY[:]
                        )
                    nc.sync.dma_start(y_ap[t * 128 : (t + 1) * 128, :], y_sb[:])

    nc.compile()
    return nc


def _get_compiled(mode):
    if mode not in _compiled:
        _compiled[mode] = _build(mode)
    return _compiled[mode]


def _prep_inputs(mode, x, Wq, bq, Wk, bk, Wv, bv, Wo, bo):
    np_mdt = np.float32 if mode == "f32r" else np.float16

    def blockdiag_lhsT(W):
        out = np.zeros((128, 128), np.float32)
        out[:64, :64] = W.T
        out[64:, 64:] = W.T
        return out.astype(np_mdt)

    wq_bd = blockdiag_lhsT(Wq)
    wk_bd = blockdiag_lhsT(Wk)
    wv_bd = blockdiag_lhsT(Wv)
    bias = np.stack(
        [np.concatenate([b, b]) for b in (bq, bk, bv)], axis=1
    ).astype(np.float32)  # [128, 3]
    woT = np.ascontiguousarray(Wo.T).astype(np_mdt)
    xT = np.ascontiguousarray(np.transpose(x, (0, 2, 1))).astype(np_mdt)  # [B, D, M]
    in_maps = [
        {
            "xT": xT[b],
            "woT": woT,
            "wq": wq_bd,
            "wk": wk_bd,
            "wv": wv_bd,
            "bias": bias,
        }
        for b in range(B)
    ]
    return in_maps


def run(inputs, trace=False, trace_kwargs=None, mode=DTYPE_MODE):
    """Run on HW; returns (full_output, BassKernelResults)."""
    from concourse.bass_utils import run_bass_kernel_spmd

    inputs = {k: np.asarray(v) for k, v in inputs.items()}
    nc = _get_compiled(mode)
    in_maps = _prep_inputs(
        mode,
        inputs["x"],
        inputs["Wq"], inputs["bq"],
        inputs["Wk"], inputs["bk"],
        inputs["Wv"], inputs["bv"],
        inputs["Wo"], inputs["bo"],
    )
    kw = dict(trace_kwargs or {})
    res = run_bass_kernel_spmd(nc, in_maps, list(range(B)), trace=trace, **kw)
    out = np.empty((B, M, D), np.float32)
    out5 = out.reshape(B, 2, 8, 64, D)  # [bo, s, b, d, Do]
    for b in range(B):
        Y = res.results[b]["y"]  # [1024(j=h*64+d), 1024(Do)]
        out5[:, :, b] = Y.reshape(8, 2, 64, D)
    out += np.asarray(inputs["bo"], np.float32)[None, None, :]
    return out, res


def kernel(**inputs):
    out, _ = run(inputs)
    return out


# revision 20
# speedup vs baseline: 1.5392x; 1.5392x over previous
"""Multi-head attention Trainium2 kernel (Bass/Tile), data-parallel over batch.

Problem shapes (hardcoded): x [8, 1024, 1024] fp32, 16 heads x 64 dim,
shared per-head projections Wq/Wk/Wv [64, 64], output proj Wo [1024, 1024].

Reference math (note quirks):
  xh = x reshaped to [h, b, m, d]
  Q/K/V = xh @ W{q,k,v}.T + b
  scores = einsum('hbmd,hbnd->hbmn', K, Q) / sqrt(1024)   (K @ Q^T!)
  A = softmax(scores, axis=-1)
  out = (A @ V) transposed (0,1,3,2) then .reshape(b, m, D) @ Wo.T + bo

Per-core plan (core b handles batch b, no collectives):
  - host prepares xT = x[b].T, blockdiag lhsT weights for 2-head packed
    projections, WoT = Wo.T
  - QT/KT/VT [64*16, m] via blockdiag [128,128] matmuls
  - per head: S_T[n, m] = QT.T @ KT (scores transposed); even/odd head
    matmuls adjacent (disjoint PE row groups); exp on ACT, scale 1/32
    (softmax max-subtraction skipped; scores are O(1))
  - U[65, m] = [V | ones].T @ expS  -> row 64 = softmax denominator
  - PE-transpose U -> [m, 65], normalize cols by reciprocal of col 64 -> P.T
  - Y rows for the pair's heads = P.T chunk.T @ WoT (bo added on host);
    host scatters Y rows (j = h*64+d) into the full output
Matmul dtype configurable: "f32r" (fp22 multiply, ~2.5e-4 rel err) or
"f16" (fp16 multiply, faster weight loads, ~5e-4 rel err).
"""

import os

import numpy as np

B = 8
M = 1024
D = 1024
NT = 8  # 128-row tiles in M / D

DTYPE_MODE = os.environ.get("KERNEL_DTYPE", "f16")

# structural knobs (sweepable via TimelineSim)
DEFAULT_CFG = dict(
    av_interleave=False,  # AV matmuls per-ntp right after exp vs end of mh
    qkv_ahead=True,       # emit QKV of pair t+1 between mh0 and mh1 of pair t
    s_bufs=2,             # score psum tiles [128,1024]
    w_bufs=2,             # weights-path psum tiles [128,512]
    u_bufs=2,             # AV accumulator psum tiles [65,512]
    es_bufs=4,            # expS sbuf tiles
    defer_tail=False,     # norm+final of pair t after attn mh0 of pair t+1
)

_compiled = {}


def _build(mode, cfg=None):
    import concourse.bacc as bacc
    import concourse.mybir as mybir
    import concourse.tile as tile
    from concourse.masks import make_identity

    cfg = dict(DEFAULT_CFG, **(cfg or {}))
    f32 = mybir.dt.float32
    mdt = mybir.dt.float32r if mode == "f32r" else mybir.dt.float16
    Exp = mybir.ActivationFunctionType.Exp

    nc = bacc.Bacc("TRN2", target_bir_lowering=False, debug=False, num_devices=B)

    xT_ap = nc.dram_tensor("xT", [D, M], mdt, kind="ExternalInput").ap()
    woT_ap = nc.dram_tensor("woT", [D, D], mdt, kind="ExternalInput").ap()
    wq_ap = nc.dram_tensor("wq", [128, 128], mdt, kind="ExternalInput").ap()
    wk_ap = nc.dram_tensor("wk", [128, 128], mdt, kind="ExternalInput").ap()
    wv_ap = nc.dram_tensor("wv", [128, 128], mdt, kind="ExternalInput").ap()
    bias_ap = nc.dram_tensor("bias", [128, 3], f32, kind="ExternalInput").ap()
    y_ap = nc.dram_tensor("y", [D, M], f32, kind="ExternalOutput").ap()

    with tile.TileContext(nc) as tc:
        with (
            tc.tile_pool(name="persist", bufs=1) as persist,
            tc.tile_pool(name="qkv", bufs=2) as qkv_pool,
            tc.tile_pool(name="vnat", bufs=2) as vnat_pool,
            tc.tile_pool(name="exps", bufs=cfg["es_bufs"]) as exps_pool,
            tc.tile_pool(name="usb", bufs=3) as usb_pool,
            tc.tile_pool(name="ysb", bufs=2) as ysb_pool,
            tc.tile_pool(name="rec", bufs=4) as rec_pool,
            tc.tile_pool(name="ps", bufs=1, space="PSUM") as ps_pool,
        ):
            # ---- persistent tiles + loads ----
            xT_all = persist.tile([128, NT * M], mdt)  # tile t at cols t*M
            woT_all = persist.tile([128, NT * D], mdt)
            PT_all = persist.tile([128, NT * D], mdt)  # [m-local, mt*D + h*64+d]
            wq_sb = persist.tile([128, 128], mdt)
            wk_sb = persist.tile([128, 128], mdt)
            wv_sb = persist.tile([128, 128], mdt)
            bias_sb = persist.tile([128, 3], f32)
            identity = persist.tile([128, 128], f32)

            with nc.named_scope("loads"):
                nc.sync.dma_start(wq_sb[:], wq_ap[:])
                nc.sync.dma_start(wk_sb[:], wk_ap[:])
                nc.sync.dma_start(wv_sb[:], wv_ap[:])
                nc.sync.dma_start(bias_sb[:], bias_ap[:])
                for t in range(NT):
                    nc.sync.dma_start(
                        xT_all[:, t * M : (t + 1) * M],
                        xT_ap[t * 128 : (t + 1) * 128, :],
                    )
                for t in range(NT):
                    nc.sync.dma_start(
                        woT_all[:, t * D : (t + 1) * D],
                        woT_ap[t * 128 : (t + 1) * 128, :],
                    )
                make_identity(nc, identity[:])

            def emit_qkv(t):
                """QKV projections + V-natural for pair t."""
                with nc.named_scope(f"qkv_p{t}"):
                    qT = qkv_pool.tile([128, M], mdt, tag="qT", name="qT")
                    kT = qkv_pool.tile([128, M], mdt, tag="kT", name="kT")
                    vT = qkv_pool.tile([128, M], f32, tag="vT", name="vT")
                    for pi, (w_sb, dest) in enumerate(
                        ((wq_sb, qT), (wk_sb, kT), (wv_sb, vT))
                    ):
                        for mh in range(2):
                            ps = ps_pool.tile(
                                [128, 512], f32, tag="w", bufs=cfg["w_bufs"],
                                name="psQKV",
                            )
                            nc.tensor.matmul(
                                ps[:],
                                w_sb[:],
                                xT_all[:, t * M + mh * 512 : t * M + (mh + 1) * 512],
                                start=True,
                                stop=True,
                            )
                            nc.vector.tensor_scalar_add(
                                dest[:, mh * 512 : (mh + 1) * 512],
                                ps[:],
                                bias_sb[:, pi : pi + 1],
                            )

                    # V natural (both heads) + ones cols:
                    # per nt block of 130: [even 64 | 1 | odd 64 | 1]
                    v_nat = vnat_pool.tile(
                        [128, NT * 130], mdt, tag="vn", name="v_nat"
                    )
                    for g in range(2):
                        pst = ps_pool.tile(
                            [128, 512], f32, tag="w", bufs=cfg["w_bufs"], name="psVT"
                        )
                        for j in range(4):
                            nt = 4 * g + j
                            nc.tensor.transpose(
                                pst[:, j * 128 : (j + 1) * 128],
                                vT[:, nt * 128 : (nt + 1) * 128],
                                identity[:],
                            )
                        vdst = v_nat[:, g * 520 : (g + 1) * 520].rearrange(
                            "p (n c) -> p n c", c=130
                        )
                        vsrc = pst[:].rearrange("p (n c) -> p n c", c=128)
                        nc.vector.tensor_copy(vdst[:, :, 0:64], vsrc[:, :, 0:64])
                        nc.vector.tensor_copy(vdst[:, :, 65:129], vsrc[:, :, 64:128])
                        ones_cast = f32 if mode == "f32r" else mdt
                        nc.gpsimd.memset(vdst[:, :, 64:65].bitcast(ones_cast), 1.0)
                        nc.gpsimd.memset(vdst[:, :, 129:130].bitcast(ones_cast), 1.0)
                return qT, kT, v_nat

            def emit_attn_mh(t, mh, qT, kT, v_nat, u_sbs):
                """Scores + exp + AV for both heads of pair t, half mh."""
                with nc.named_scope(f"attn_p{t}_m{mh}"):
                    expS = [
                        exps_pool.tile(
                            [128, NT * 512], mdt, tag="es", name="expS_e"
                        ),
                        exps_pool.tile(
                            [128, NT * 512], mdt, tag="es", name="expS_o"
                        ),
                    ]
                    psU = [None, None]

                    def get_psU(hh):
                        if psU[hh] is None:
                            psU[hh] = ps_pool.tile(
                                [65, 512], f32, tag="u", bufs=cfg["u_bufs"],
                                name="psU",
                            )
                        return psU[hh]

                    def av(nt, hh_list=(0, 1)):
                        for hh in hh_list:
                            o = nt * 130 + hh * 65
                            nc.tensor.matmul(
                                get_psU(hh)[:],
                                v_nat[:, o : o + 65],
                                expS[hh][:, nt * 512 : (nt + 1) * 512],
                                start=(nt == 0),
                                stop=(nt == NT - 1),
                            )

                    for ntp in range(4):
                        psS = [
                            ps_pool.tile(
                                [128, 1024], f32, tag="s", bufs=cfg["s_bufs"],
                                name="psS_e",
                            ),
                            ps_pool.tile(
                                [128, 1024], f32, tag="s", bufs=cfg["s_bufs"],
                                name="psS_o",
                            ),
                        ]
                        for sub in range(2):
                            nt = 2 * ntp + sub
                            for hh in range(2):
                                part = hh * 64
                                nc.tensor.matmul(
                                    psS[hh][:, sub * 512 : (sub + 1) * 512],
                                    qT[part : part + 64, nt * 128 : (nt + 1) * 128],
                                    kT[part : part + 64, mh * 512 : (mh + 1) * 512],
                                    start=True,
                                    stop=True,
                                )
                        for hh in range(2):
                            nc.scalar.activation(
                                expS[hh][:, ntp * 1024 : (ntp + 1) * 1024],
                                psS[hh][:],
                                Exp,
                                scale=1.0 / 32.0,
                            )
                        if cfg["av_interleave"]:
                            av(2 * ntp)
                            av(2 * ntp + 1)
                    if not cfg["av_interleave"]:
                        for hh in range(2):
                            for nt in range(NT):
                                av(nt, hh_list=(hh,))
                            nc.vector.tensor_copy(
                                u_sbs[hh][:, mh * 512 : (mh + 1) * 512],
                                psU[hh][:],
                            )
                    else:
                        for hh in range(2):
                            nc.vector.tensor_copy(
                                u_sbs[hh][:, mh * 512 : (mh + 1) * 512],
                                psU[hh][:],
                            )

            def emit_norm(t, u_sbs):
                """Transpose+normalize U into PT_all for both heads of pair t."""
                for hh in range(2):
                    h = 2 * t + hh
                    u_sb = u_sbs[hh]
                    with nc.named_scope(f"norm_h{h}"):
                        pstUs = []
                        rec = rec_pool.tile([128, NT], f32, tag="r", name="rec")
                        for g in range(2):
                            pstU = ps_pool.tile(
                                [128, 512], f32, tag="w", bufs=cfg["w_bufs"],
                                name="pstU",
                            )
                            pstUs.append(pstU)
                            for j in range(4):
                                mt = 4 * g + j
                                nc.tensor.transpose(
                                    pstU[:, j * 128 : j * 128 + 65],
                                    u_sb[:, mt * 128 : (mt + 1) * 128],
                                    identity[:65, :65],
                                )
                            nc.vector.tensor_copy(
                                rec[:, g * 4 : (g + 1) * 4],
                                pstU[:]
                                .rearrange("p (n c) -> p n c", c=128)[:, :, 64:65]
                                .rearrange("p n c -> p (n c)"),
                            )
                        nc.vector.reciprocal(rec[:], rec[:])
                        for g in range(2):
                            for j in range(4):
                                mt = 4 * g + j
                                nc.vector.tensor_scalar_mul(
                                    PT_all[
                                        :, mt * D + h * 64 : mt * D + h * 64 + 64
                                    ],
                                    pstUs[g][:, j * 128 : j * 128 + 64],
                                    rec[:, mt : mt + 1],
                                )

            def emit_final(t):
                """Output-projection rows for pair t (j = 128t..128t+127)."""
                with nc.named_scope(f"final_p{t}"):
                    y_sb = ysb_pool.tile([128, 1024], f32, tag="y", name="y_sb")
                    for dh in range(2):
                        psY = ps_pool.tile(
                            [128, 512], f32, tag="w", bufs=cfg["w_bufs"], name="psY"
                        )
                        for mt in range(NT):
                            nc.tensor.matmul(
                                psY[:],
                                PT_all[:, mt * D + t * 128 : mt * D + (t + 1) * 128],
                                woT_all[
                                    :, mt * D + dh * 512 : mt * D + (dh + 1) * 512
                                ],
                                start=(mt == 0),
                                stop=(mt == NT - 1),
                            )
                        nc.vector.tensor_copy(
                            y_sb[:, dh * 512 : (dh + 1) * 512], psY[:]
                        )
                    nc.sync.dma_start(y_ap[t * 128 : (t + 1) * 128, :], y_sb[:])

            # ---- pair loop (software-pipelined when qkv_ahead) ----
            if cfg["qkv_ahead"]:
                cur = emit_qkv(0)
                pend = None  # (t, u_sbs) awaiting norm+final
                for t in range(8):
                    u_sbs = [
                        usb_pool.tile([65, M], f32, tag="u", name="u_sb")
                        for _ in range(2)
                    ]
                    qT, kT, v_nat = cur
                    emit_attn_mh(t, 0, qT, kT, v_nat, u_sbs)
                    if cfg.get("defer_tail") and pend is not None:
                        emit_norm(*pend)
                        emit_final(pend[0])
                    if t + 1 < 8:
                        cur = emit_qkv(t + 1)
                    emit_attn_mh(t, 1, qT, kT, v_nat, u_sbs)
                    if cfg.get("defer_tail"):
                        pend = (t, u_sbs)
                    else:
                        emit_norm(t, u_sbs)
                        emit_final(t)
                if cfg.get("defer_tail") and pend is not None:
                    emit_norm(*pend)
                    emit_final(pend[0])
            else:
                for t in range(8):
                    u_sbs = [
                        usb_pool.tile([65, M], f32, tag="u", name="u_sb")
                        for _ in range(2)
                    ]
                    qT, kT, v_nat = emit_qkv(t)
                    emit_attn_mh(t, 0, qT, kT, v_nat, u_sbs)
                    emit_attn_mh(t, 1, qT, kT, v_nat, u_sbs)
                    emit_norm(t, u_sbs)
                    emit_final(t)

    nc.compile()
    return nc


def _get_compiled(mode):
    if mode not in _compiled:
        _compiled[mode] = _build(mode)
    return _compiled[mode]


def _prep_inputs(mode, x, Wq, bq, Wk, bk, Wv, bv, Wo, bo):
    np_mdt = np.float32 if mode == "f32r" else np.float16

    def blockdiag_lhsT(W):
        out = np.zeros((128, 128), np.float32)
        out[:64, :64] = W.T
        out[64:, 64:] = W.T
        return out.astype(np_mdt)

    wq_bd = blockdiag_lhsT(Wq)
    wk_bd = blockdiag_lhsT(Wk)
    wv_bd = blockdiag_lhsT(Wv)
    bias = np.stack(
        [np.concatenate([b, b]) for b in (bq, bk, bv)], axis=1
    ).astype(np.float32)  # [128, 3]
    woT = np.ascontiguousarray(Wo.T).astype(np_mdt)
    xT = np.ascontiguousarray(np.transpose(x, (0, 2, 1))).astype(np_mdt)  # [B,D,M]
    in_maps = [
        {
            "xT": xT[b],
            "woT": woT,
            "wq": wq_bd,
            "wk": wk_bd,
            "wv": wv_bd,
            "bias": bias,
        }
        for b in range(B)
    ]
    return in_maps


def run(inputs, trace=False, trace_kwargs=None, mode=DTYPE_MODE):
    """Run on HW; returns (full_output, BassKernelResults)."""
    from concourse.bass_utils import run_bass_kernel_spmd

    inputs = {k: np.asarray(v) for k, v in inputs.items()}
    nc = _get_compiled(mode)
    in_maps = _prep_inputs(
        mode,
        inputs["x"],
        inputs["Wq"], inputs["bq"],
        inputs["Wk"], inputs["bk"],
        inputs["Wv"], inputs["bv"],
        inputs["Wo"], inputs["bo"],
    )
    kw = dict(trace_kwargs or {})
    res = run_bass_kernel_spmd(nc, in_maps, list(range(B)), trace=trace, **kw)
    out = np.empty((B, M, D), np.float32)
    out5 = out.reshape(B, 2, 8, 64, D)  # [bo, s, b, d, Do]
    for b in range(B):
        Y = res.results[b]["y"]  # [1024(j=h*64+d), 1024(Do)]
        out5[:, :, b] = Y.reshape(8, 2, 64, D)
    out += np.asarray(inputs["bo"], np.float32)[None, None, :]
    return out, res


def kernel(**inputs):
    out, _ = run(inputs)
    return out


# revision 21
# speedup vs baseline: 1.6296x; 1.0587x over previous
"""Multi-head attention Trainium2 kernel (Bass/Tile), data-parallel over batch.

Problem shapes (hardcoded): x [8, 1024, 1024] fp32, 16 heads x 64 dim,
shared per-head projections Wq/Wk/Wv [64, 64], output proj Wo [1024, 1024].

Reference math (note quirks):
  xh = x reshaped to [h, b, m, d]
  Q/K/V = xh @ W{q,k,v}.T + b
  scores = einsum('hbmd,hbnd->hbmn', K, Q) / sqrt(1024)   (K @ Q^T!)
  A = softmax(scores, axis=-1)
  out = (A @ V) transposed (0,1,3,2) then .reshape(b, m, D) @ Wo.T + bo

Per-core plan (core b handles batch b, no collectives):
  - host prepares xT = x[b].T, blockdiag lhsT weights for 2-head packed
    projections, WoT = Wo.T
  - QT/KT/VT [64*16, m] via blockdiag [128,128] matmuls
  - per head: S_T[n, m] = QT.T @ KT (scores transposed); even/odd head
    matmuls adjacent (disjoint PE row groups); exp on ACT, scale 1/32
    (softmax max-subtraction skipped; scores are O(1))
  - U[65, m] = [V | ones].T @ expS  -> row 64 = softmax denominator
  - PE-transpose U -> [m, 65], normalize cols by reciprocal of col 64 -> P.T
  - Y rows for the pair's heads = P.T chunk.T @ WoT (bo added on host);
    host scatters Y rows (j = h*64+d) into the full output
Matmul dtype configurable: "f32r" (fp22 multiply, ~2.5e-4 rel err) or
"f16" (fp16 multiply, faster weight loads, ~5e-4 rel err).
"""

import os

import numpy as np

B = 8
M = 1024
D = 1024
NT = 8  # 128-row tiles in M / D

DTYPE_MODE = os.environ.get("KERNEL_DTYPE", "f16")

# structural knobs (sweepable via TimelineSim)
DEFAULT_CFG = dict(
    av_interleave=False,  # AV matmuls per-ntp right after exp vs end of mh
    qkv_ahead=True,       # emit QKV of pair t+1 between mh0 and mh1 of pair t
    s_bufs=2,             # score psum tiles [128,1024]
    w_bufs=2,             # weights-path psum tiles [128,512]
    u_bufs=2,             # AV accumulator psum tiles [65,512]
    es_bufs=4,            # expS sbuf tiles
    defer_tail=False,     # norm+final of pair t after attn mh0 of pair t+1
)

_compiled = {}


def _build(mode, cfg=None):
    import concourse.bacc as bacc
    import concourse.mybir as mybir
    import concourse.tile as tile
    from concourse.masks import make_identity

    cfg = dict(DEFAULT_CFG, **(cfg or {}))
    f32 = mybir.dt.float32
    mdt = mybir.dt.float32r if mode == "f32r" else mybir.dt.float16
    tdt = f32 if mode == "f32r" else mdt  # transpose-path dtype
    Exp = mybir.ActivationFunctionType.Exp

    nc = bacc.Bacc("TRN2", target_bir_lowering=False, debug=False, num_devices=B)

    xT_ap = nc.dram_tensor("xT", [D, M], mdt, kind="ExternalInput").ap()
    woT_ap = nc.dram_tensor("woT", [D, D], mdt, kind="ExternalInput").ap()
    wq_ap = nc.dram_tensor("wq", [128, 128], mdt, kind="ExternalInput").ap()
    wk_ap = nc.dram_tensor("wk", [128, 128], mdt, kind="ExternalInput").ap()
    wv_ap = nc.dram_tensor("wv", [128, 128], mdt, kind="ExternalInput").ap()
    bias_ap = nc.dram_tensor("bias", [128, 3], f32, kind="ExternalInput").ap()
    y_ap = nc.dram_tensor("y", [D, M], f32, kind="ExternalOutput").ap()

    with tile.TileContext(nc) as tc:
        with (
            tc.tile_pool(name="persist", bufs=1) as persist,
            tc.tile_pool(name="qkv", bufs=2) as qkv_pool,
            tc.tile_pool(name="vnat", bufs=2) as vnat_pool,
            tc.tile_pool(name="exps", bufs=cfg["es_bufs"]) as exps_pool,
            tc.tile_pool(name="usb", bufs=3) as usb_pool,
            tc.tile_pool(name="ysb", bufs=2) as ysb_pool,
            tc.tile_pool(name="rec", bufs=4) as rec_pool,
            tc.tile_pool(name="ps", bufs=1, space="PSUM") as ps_pool,
        ):
            # ---- persistent tiles + loads ----
            xT_all = persist.tile([128, NT * M], mdt)  # tile t at cols t*M
            woT_all = persist.tile([128, NT * D], mdt)
            PT_all = persist.tile([128, NT * D], mdt)  # [m-local, mt*D + h*64+d]
            wq_sb = persist.tile([128, 128], mdt)
            wk_sb = persist.tile([128, 128], mdt)
            wv_sb = persist.tile([128, 128], mdt)
            bias_sb = persist.tile([128, 3], f32)
            identity = persist.tile([128, 128], tdt)

            with nc.named_scope("loads"):
                nc.sync.dma_start(wq_sb[:], wq_ap[:])
                nc.sync.dma_start(wk_sb[:], wk_ap[:])
                nc.sync.dma_start(wv_sb[:], wv_ap[:])
                nc.sync.dma_start(bias_sb[:], bias_ap[:])
                for t in range(NT):
                    for half in range(2):
                        nc.sync.dma_start(
                            xT_all[:, t * M + half * 512 : t * M + (half + 1) * 512],
                            xT_ap[t * 128 : (t + 1) * 128, half * 512 : (half + 1) * 512],
                        )
                for t in range(NT):
                    nc.sync.dma_start(
                        woT_all[:, t * D : (t + 1) * D],
                        woT_ap[t * 128 : (t + 1) * 128, :],
                    )
                make_identity(nc, identity[:])

            def emit_qkv(t):
                """QKV projections + V-natural for pair t."""
                with nc.named_scope(f"qkv_p{t}"):
                    qT = qkv_pool.tile([128, M], mdt, tag="qT", name="qT")
                    kT = qkv_pool.tile([128, M], mdt, tag="kT", name="kT")
                    vT = qkv_pool.tile([128, M], tdt, tag="vT", name="vT")
                    for pi, (w_sb, dest) in enumerate(
                        ((wq_sb, qT), (wk_sb, kT), (wv_sb, vT))
                    ):
                        for mh in range(2):
                            ps = ps_pool.tile(
                                [128, 512], f32, tag="w", bufs=cfg["w_bufs"],
                                name="psQKV",
                            )
                            nc.tensor.matmul(
                                ps[:],
                                w_sb[:],
                                xT_all[:, t * M + mh * 512 : t * M + (mh + 1) * 512],
                                start=True,
                                stop=True,
                            )
                            nc.vector.tensor_scalar_add(
                                dest[:, mh * 512 : (mh + 1) * 512],
                                ps[:],
                                bias_sb[:, pi : pi + 1],
                            )

                    # V natural (both heads) + ones cols:
                    # per nt block of 130: [even 64 | 1 | odd 64 | 1]
                    v_nat = vnat_pool.tile(
                        [128, NT * 130], mdt, tag="vn", name="v_nat"
                    )
                    for g in range(2):
                        pst = ps_pool.tile(
                            [128, 512], tdt, tag="w", bufs=cfg["w_bufs"], name="psVT"
                        )
                        for j in range(4):
                            nt = 4 * g + j
                            nc.tensor.transpose(
                                pst[:, j * 128 : (j + 1) * 128],
                                vT[:, nt * 128 : (nt + 1) * 128],
                                identity[:],
                            )
                        vdst = v_nat[:, g * 520 : (g + 1) * 520].rearrange(
                            "p (n c) -> p n c", c=130
                        )
                        vsrc = pst[:].rearrange("p (n c) -> p n c", c=128)
                        nc.vector.tensor_copy(vdst[:, :, 0:64], vsrc[:, :, 0:64])
                        nc.vector.tensor_copy(vdst[:, :, 65:129], vsrc[:, :, 64:128])
                        ones_cast = f32 if mode == "f32r" else mdt
                        nc.gpsimd.memset(vdst[:, :, 64:65].bitcast(ones_cast), 1.0)
                        nc.gpsimd.memset(vdst[:, :, 129:130].bitcast(ones_cast), 1.0)
                return qT, kT, v_nat

            def emit_attn_mh(t, mh, qT, kT, v_nat, u_sbs):
                """Scores + exp + AV for both heads of pair t, half mh."""
                with nc.named_scope(f"attn_p{t}_m{mh}"):
                    expS = [
                        exps_pool.tile(
                            [128, NT * 512], mdt, tag="es", name="expS_e"
                        ),
                        exps_pool.tile(
                            [128, NT * 512], mdt, tag="es", name="expS_o"
                        ),
                    ]
                    psU = [None, None]

                    def get_psU(hh):
                        if psU[hh] is None:
                            psU[hh] = ps_pool.tile(
                                [65, 512], f32, tag="u", bufs=cfg["u_bufs"],
                                name="psU",
                            )
                        return psU[hh]

                    def av(nt, hh_list=(0, 1)):
                        for hh in hh_list:
                            o = nt * 130 + hh * 65
                            nc.tensor.matmul(
                                get_psU(hh)[:],
                                v_nat[:, o : o + 65],
                                expS[hh][:, nt * 512 : (nt + 1) * 512],
                                start=(nt == 0),
                                stop=(nt == NT - 1),
                            )

                    for ntp in range(4):
                        psS = [
                            ps_pool.tile(
                                [128, 1024], f32, tag="s", bufs=cfg["s_bufs"],
                                name="psS_e",
                            ),
                            ps_pool.tile(
                                [128, 1024], f32, tag="s", bufs=cfg["s_bufs"],
                                name="psS_o",
                            ),
                        ]
                        for sub in range(2):
                            nt = 2 * ntp + sub
                            for hh in range(2):
                                part = hh * 64
                                nc.tensor.matmul(
                                    psS[hh][:, sub * 512 : (sub + 1) * 512],
                                    qT[part : part + 64, nt * 128 : (nt + 1) * 128],
                                    kT[part : part + 64, mh * 512 : (mh + 1) * 512],
                                    start=True,
                                    stop=True,
                                )
                        for hh in range(2):
                            nc.scalar.activation(
                                expS[hh][:, ntp * 1024 : (ntp + 1) * 1024],
                                psS[hh][:],
                                Exp,
                                scale=1.0 / 32.0,
                            )
                        if cfg["av_interleave"]:
                            av(2 * ntp)
                            av(2 * ntp + 1)
                    if not cfg["av_interleave"]:
                        for hh in range(2):
                            for nt in range(NT):
                                av(nt, hh_list=(hh,))
                            nc.vector.tensor_copy(
                                u_sbs[hh][:, mh * 512 : (mh + 1) * 512],
                                psU[hh][:],
                            )
                    else:
                        for hh in range(2):
                            nc.vector.tensor_copy(
                                u_sbs[hh][:, mh * 512 : (mh + 1) * 512],
                                psU[hh][:],
                            )

            def emit_norm(t, u_sbs):
                """Transpose+normalize U into PT_all for both heads of pair t."""
                for hh in range(2):
                    h = 2 * t + hh
                    u_sb = u_sbs[hh]
                    with nc.named_scope(f"norm_h{h}"):
                        pstUs = []
                        rec = rec_pool.tile([128, NT], f32, tag="r", name="rec")
                        for g in range(2):
                            pstU = ps_pool.tile(
                                [128, 512], tdt, tag="w", bufs=cfg["w_bufs"],
                                name="pstU",
                            )
                            pstUs.append(pstU)
                            for j in range(4):
                                mt = 4 * g + j
                                nc.tensor.transpose(
                                    pstU[:, j * 128 : j * 128 + 65],
                                    u_sb[:, mt * 128 : (mt + 1) * 128],
                                    identity[:65, :65],
                                )
                            nc.vector.tensor_copy(
                                rec[:, g * 4 : (g + 1) * 4],
                                pstU[:]
                                .rearrange("p (n c) -> p n c", c=128)[:, :, 64:65]
                                .rearrange("p n c -> p (n c)"),
                            )
                        nc.vector.reciprocal(rec[:], rec[:])
                        for g in range(2):
                            for j in range(4):
                                mt = 4 * g + j
                                nc.vector.tensor_scalar_mul(
                                    PT_all[
                                        :, mt * D + h * 64 : mt * D + h * 64 + 64
                                    ],
                                    pstUs[g][:, j * 128 : j * 128 + 64],
                                    rec[:, mt : mt + 1],
                                )

            def emit_final(t):
                """Output-projection rows for pair t (j = 128t..128t+127)."""
                with nc.named_scope(f"final_p{t}"):
                    y_sb = ysb_pool.tile([128, 1024], f32, tag="y", name="y_sb")
                    for dh in range(2):
                        psY = ps_pool.tile(
                            [128, 512], f32, tag="w", bufs=cfg["w_bufs"], name="psY"
                        )
                        for mt in range(NT):
                            nc.tensor.matmul(
                                psY[:],
                                PT_all[:, mt * D + t * 128 : mt * D + (t + 1) * 128],
                                woT_all[
                                    :, mt * D + dh * 512 : mt * D + (dh + 1) * 512
                                ],
                                start=(mt == 0),
                                stop=(mt == NT - 1),
                            )
                        nc.vector.tensor_copy(
                            y_sb[:, dh * 512 : (dh + 1) * 512], psY[:]
                        )
                    nc.sync.dma_start(y_ap[t * 128 : (t + 1) * 128, :], y_sb[:])

            # ---- pair loop (software-pipelined when qkv_ahead) ----
            if cfg["qkv_ahead"]:
                cur = emit_qkv(0)
                pend = None  # (t, u_sbs) awaiting norm+final
                for t in range(8):
                    u_sbs = [
                        usb_pool.tile([65, M], tdt, tag="u", name="u_sb")
                        for _ in range(2)
                    ]
                    qT, kT, v_nat = cur
                    emit_attn_mh(t, 0, qT, kT, v_nat, u_sbs)
                    if cfg.get("defer_tail") and pend is not None:
                        emit_norm(*pend)
                        emit_final(pend[0])
                    if t + 1 < 8:
                        cur = emit_qkv(t + 1)
                    emit_attn_mh(t, 1, qT, kT, v_nat, u_sbs)
                    if cfg.get("defer_tail"):
                        pend = (t, u_sbs)
                    else:
                        emit_norm(t, u_sbs)
                        emit_final(t)
                if cfg.get("defer_tail") and pend is not None:
                    emit_norm(*pend)
                    emit_final(pend[0])
            else:
                for t in range(8):
                    u_sbs = [
                        usb_pool.tile([65, M], tdt, tag="u", name="u_sb")
                        for _ in range(2)
                    ]
                    qT, kT, v_nat = emit_qkv(t)
                    emit_attn_mh(t, 0, qT, kT, v_nat, u_sbs)
                    emit_attn_mh(t, 1, qT, kT, v_nat, u_sbs)
                    emit_norm(t, u_sbs)
                    emit_final(t)

    nc.compile()
    return nc


def _get_compiled(mode):
    if mode not in _compiled:
        _compiled[mode] = _build(mode)
    return _compiled[mode]


def _prep_inputs(mode, x, Wq, bq, Wk, bk, Wv, bv, Wo, bo):
    np_mdt = np.float32 if mode == "f32r" else np.float16

    def blockdiag_lhsT(W):
        out = np.zeros((128, 128), np.float32)
        out[:64, :64] = W.T
        out[64:, 64:] = W.T
        return out.astype(np_mdt)

    wq_bd = blockdiag_lhsT(Wq)
    wk_bd = blockdiag_lhsT(Wk)
    wv_bd = blockdiag_lhsT(Wv)
    bias = np.stack(
        [np.concatenate([b, b]) for b in (bq, bk, bv)], axis=1
    ).astype(np.float32)  # [128, 3]
    woT = np.ascontiguousarray(Wo.T).astype(np_mdt)
    xT = np.ascontiguousarray(np.transpose(x, (0, 2, 1))).astype(np_mdt)  # [B,D,M]
    in_maps = [
        {
            "xT": xT[b],
            "woT": woT,
            "wq": wq_bd,
            "wk": wk_bd,
            "wv": wv_bd,
            "bias": bias,
        }
        for b in range(B)
    ]
    return in_maps


def run(inputs, trace=False, trace_kwargs=None, mode=DTYPE_MODE):
    """Run on HW; returns (full_output, BassKernelResults)."""
    from concourse.bass_utils import run_bass_kernel_spmd

    inputs = {k: np.asarray(v) for k, v in inputs.items()}
    nc = _get_compiled(mode)
    in_maps = _prep_inputs(
        mode,
        inputs["x"],
        inputs["Wq"], inputs["bq"],
        inputs["Wk"], inputs["bk"],
        inputs["Wv"], inputs["bv"],
        inputs["Wo"], inputs["bo"],
    )
    kw = dict(trace_kwargs or {})
    res = run_bass_kernel_spmd(nc, in_maps, list(range(B)), trace=trace, **kw)
    out = np.empty((B, M, D), np.float32)
    out5 = out.reshape(B, 2, 8, 64, D)  # [bo, s, b, d, Do]
    for b in range(B):
        Y = res.results[b]["y"]  # [1024(j=h*64+d), 1024(Do)]
        out5[:, :, b] = Y.reshape(8, 2, 64, D)
    out += np.asarray(inputs["bo"], np.float32)[None, None, :]
    return out, res


def kernel(**inputs):
    out, _ = run(inputs)
    return out
